# revision 10
# baseline (speedup 1.0000x reference)
"""Llama GQA attention (B=2, S=2048, H=4096, 32 q heads / 8 kv heads, HD=128)
on 8 Trainium2 NeuronCores.

Sharding: DP=2 over batch x TP=4 over heads.
  core c: batch b = c // 4, tp rank r = c % 4
  - owns q heads [8r, 8r+8), kv heads [2r, 2r+2)
  - AllGather (bf16) of attention outputs within each 4-core batch group,
    split in two 4-head pieces triggered mid-attention
  - output projection over the full 4096 attn features for output columns
    [1024r, 1024(r+1)) -> disjoint outputs, host concatenates.

All inputs are pre-cast/pre-transposed to the on-chip layout on the HOST
(bf16 weights/X^T), so the device does zero dtype conversion and reads
every operand exactly once per use:
  xt:  [128, kt*S + t]   X^T with hidden dim split into 32 k-tiles
  w:   [128, panel*4096] weight panels (8 wq, 2 wk, 1 double-width wv,
                          8 wo), each [128 k-part, kt*cols + c]
On-chip layout is "transposed" ([feature, token]) for Q/K; V is computed
DIRECTLY in [token, dim] layout (lhsT = X^T tile stationary, wv panel
moving, N=256 covering both kv heads) -- no PE transposes needed.
Causal masking: per q-chunk c only k-tiles 0..4c+3 are touched; the 4
diagonal tiles compute only the surviving q-range (free = 512-128j) plus
one static [128,128] triangular mask on the leading square (GpSimd).
Softmax skips max-subtraction (scores are O(8), exp fits bf16). The
denominator merges each adjacent pair of exp tiles with one in-place
DVE bf16 add and runs a single ones-matmul per pair on the PE (half
the denominator matmuls); the reciprocal uses the fast DVE
approximation (18 bits).

Schedule (v2): chunk-interleaved -- proj(c) then IMMEDIATELY attn(c)
(possible because attn(c) only needs K/V chunks 0..c), so the first
AllGather piece triggers ~120us into the kernel instead of ~480us.
The 8 AllGather pieces serialize on the collective stream (~25-45us
each, ~90us for the cold first one); triggering them early hides the
entire ~350us chain behind remaining proj/attn/outproj matmul work
instead of exposing its tail. Out-projections are placed as PE filler:
op0 after attn1, op1 after attn2, op2+op3 after attn3. The af loads
carry tile_wait_until floors approximating real collective completion
so the list scheduler (whose sim has no collective-latency model) does
not hoist af-consuming matmuls ahead of available attention work.
Startup DMAs are posted xt-first across four engine queues so the
first matmul issues ~12us in instead of ~30us.
"""

import sys

for _p in ("/opt/trn_rl_repo",):
    if _p not in sys.path:
        sys.path.append(_p)

import numpy as np
import ml_dtypes

import concourse.bacc as bacc
import concourse.mybir as mybir
import concourse.tile as tile
from concourse.bass_utils import run_bass_kernel_spmd

F32 = mybir.dt.float32
BF16 = mybir.dt.bfloat16

B, S, H = 2, 2048, 4096
NH, NKV, HD = 32, 8, 128
N_CORES = 8
TP = 4
GROUPS = [[0, 1, 2, 3], [4, 5, 6, 7]]

HL = NH // TP          # 8 local q heads
KVL = NKV // TP        # 2 local kv heads
QCOLS = HL * HD        # 1024 local q cols
OC = H // TP           # 1024 local out cols

TC = 512               # token chunk (= one attention q-block)
NCHUNK = S // TC       # 4
KT = H // 128          # 32 contraction tiles
SCALE = float(HD ** -0.5)
NPANEL = 20            # 8 wq + 2 wk + 2 (double wv) + 8 wo
PANW = KT * 128        # 4096 free cols per standard weight panel

LAST_RESULT = None
_BUILT = {}


def _build():
    nc = bacc.Bacc("TRN2", debug=False, num_devices=N_CORES)

    xt_d = nc.dram_tensor("xt", [128, KT * S], BF16, kind="ExternalInput").ap()
    w_d = nc.dram_tensor("w_all", [128, NPANEL * PANW], BF16,
                         kind="ExternalInput").ap()
    cos_d = nc.dram_tensor("cos_t", [HD, S], F32, kind="ExternalInput").ap()
    sin_d = nc.dram_tensor("sin_t", [HD, S], F32, kind="ExternalInput").ap()
    mask_d = nc.dram_tensor("maskb", [128, 128], BF16, kind="ExternalInput").ap()
    ones_d = nc.dram_tensor("onesb", [128, 128], BF16, kind="ExternalInput").ap()
    out_d = nc.dram_tensor("out_t", [OC, S], F32, kind="ExternalOutput").ap()

    with tile.TileContext(nc) as tc:
        with tc.tile_pool(name="sb", bufs=1) as sb, \
             tc.tile_pool(name="ps", bufs=1, space="PSUM") as ps, \
             tc.tile_pool(name="dr", bufs=1, space="DRAM") as dr:

            # ---- persistent tiles ----
            cos_sb = sb.tile([HD, S], F32)
            sin_sb = sb.tile([HD, S], F32)
            mask_sb = sb.tile([128, 128], BF16)
            ones_sb = sb.tile([128, 128], BF16)
            ktb = sb.tile([128, KVL * S], BF16)            # roped K^T [d, kv*S+t]
            vb = sb.tile([128, (S // 128) * KVL * 128], BF16)  # V [t, tt*256+d]

            _XTB = {}

            def load_xt(c, engs=None):
                """X^T chunk c -> SBUF [128, kt*TC], 4 DMAs (spread queues)."""
                xtb = sb.tile([128, KT * TC], BF16, tag="xtb", bufs=1,
                              name=f"xtb{c}")
                engs = engs or [nc.sync] * 4
                for q in range(4):
                    k0, k1 = q * (KT // 4), (q + 1) * (KT // 4)
                    engs[q].dma_start(
                        xtb.rearrange("p (kt t) -> p kt t", t=TC)[:, k0:k1],
                        xt_d.rearrange("p (kt t) -> p kt t", t=S)
                        [:, k0:k1, c * TC:(c + 1) * TC])
                _XTB[c] = xtb

            def get_panel(idx, eng=None):
                wb = sb.tile([128, PANW], BF16, tag="wb", bufs=3, name="wb")
                (eng or nc.sync).dma_start(
                    wb[:], w_d[:, idx * PANW:(idx + 1) * PANW])
                return wb

            def get_wv2(eng=None):
                """Double-width V panel [128, kt*256 + d] at slot 10."""
                wb = sb.tile([128, 2 * PANW], BF16, tag="wv2", bufs=1,
                             name="wv2")
                (eng or nc.sync).dma_start(
                    wb[:], w_d[:, 10 * PANW:12 * PANW])
                return wb

            def rope(dst, pq, t0):
                """dst (bf16 [128, TC]) = rope of pq (fp32 PSUM [128, TC])."""
                qf = sb.tile([128, TC], F32, tag="qf", bufs=2)
                nc.scalar.copy(qf[:], pq[:])
                qs = sb.tile([128, TC], F32, tag="qs", bufs=2)
                nc.sync.dma_start(qs[0:64, :], qf[64:128, :])
                nc.sync.dma_start(qs[64:128, :], qf[0:64, :])
                nc.vector.tensor_tensor(
                    qf[:], qf[:], cos_sb[:, t0:t0 + TC], mybir.AluOpType.mult)
                nc.vector.tensor_tensor(
                    qs[:], qs[:], sin_sb[:, t0:t0 + TC], mybir.AluOpType.mult)
                nc.vector.tensor_tensor(dst, qf[:], qs[:], mybir.AluOpType.add)

            def proj(c):
                t0 = c * TC
                xtb = _XTB[c]
                qtb = sb.tile([128, HL * TC], BF16, tag="qt", bufs=2,
                              name="qtb")
                wv = get_wv2(nc.gpsimd)   # post early on an idle DMA queue
                for h in range(HL):
                    wb = get_panel(h)
                    pq = ps.tile([128, TC], F32, tag="pj", bufs=2, name="pq")
                    for kt in range(KT):
                        nc.tensor.matmul(
                            pq[:], wb[:, kt * 128:(kt + 1) * 128],
                            xtb[:, kt * TC:(kt + 1) * TC],
                            start=(kt == 0), stop=(kt == KT - 1))
                    rope(qtb[:, h * TC:(h + 1) * TC], pq, t0)
                for kv in range(KVL):
                    wb = get_panel(8 + kv)
                    pk = ps.tile([128, TC], F32, tag="pj", bufs=2, name="pk")
                    for kt in range(KT):
                        nc.tensor.matmul(
                            pk[:], wb[:, kt * 128:(kt + 1) * 128],
                            xtb[:, kt * TC:(kt + 1) * TC],
                            start=(kt == 0), stop=(kt == KT - 1))
                    rope(ktb[:, kv * S + t0:kv * S + t0 + TC], pk, t0)
                # direct V: out[t, d] accumulated with X^T tile stationary,
                # wv panel moving (N=256 covers both kv heads). Two token
                # tiles share one PSUM bank as independent 256-col groups.
                for tp_ in range(2):
                    pvd = ps.tile([128, TC], F32, tag="pj", bufs=2, name="pvd")
                    for half in range(2):
                        tb = 2 * tp_ + half
                        for kt in range(KT):
                            nc.tensor.matmul(
                                pvd[:, half * 256:(half + 1) * 256],
                                xtb[:, kt * TC + tb * 128:
                                    kt * TC + tb * 128 + 128],
                                wv[:, kt * 256:(kt + 1) * 256],
                                start=(kt == 0), stop=(kt == KT - 1))
                    vt0 = 4 * c + 2 * tp_
                    nc.scalar.copy(
                        vb[:, vt0 * 256:(vt0 + 2) * 256], pvd[:])
                return qtb

            def attention(c, qtb):
                nkt = 4 * c + 4
                attnb = sb.tile([128, HL * TC], BF16, tag="attn", bufs=2,
                                name="attnb")
                ccos = []
                for h in range(HL):
                    kv = h // (HL // KVL)
                    qsl = qtb[:, h * TC:(h + 1) * TC]
                    pa = ps.tile([128, TC], F32, tag="pa", bufs=2, name="pa")
                    # denominator accumulator (bf16 SBUF): exp tiles are
                    # summed here by DVE adds; ONE ones-matmul per head at
                    # the end replaces per-pair denominator matmuls on PE.
                    dacc = sb.tile([128, TC], BF16, tag="dacc", bufs=2,
                                   name="dacc")
                    pts = {}

                    def qoff(kt):
                        j = kt - 4 * c
                        return 128 * j if j >= 0 else 0

                    def qk_exp(kt):
                        o = qoff(kt)
                        sps = ps.tile([128, TC], F32, tag="s", bufs=2,
                                      name="sps")
                        nc.tensor.matmul(
                            sps[:, o:],
                            ktb[:, kv * S + kt * 128:kv * S + (kt + 1) * 128],
                            qsl[:, o:], start=True, stop=True)
                        pt = sb.tile([128, TC], BF16, tag="pt", bufs=7,
                                     name="pt")
                        nc.scalar.activation(
                            pt[:, o:], sps[:, o:],
                            mybir.ActivationFunctionType.Exp, scale=SCALE)
                        if kt - 4 * c >= 0:
                            nc.gpsimd.tensor_tensor(
                                pt[:, o:o + 128], pt[:, o:o + 128], mask_sb[:],
                                mybir.AluOpType.mult)
                        pts[kt] = pt

                    def pv(kt):
                        o = qoff(kt)
                        nc.tensor.matmul(
                            pa[:, o:],
                            vb[:, kt * (KVL * 128) + kv * 128:
                               kt * (KVL * 128) + (kv + 1) * 128],
                            pts[kt][:, o:],
                            start=(kt == 0), stop=(kt == nkt - 1))

                    def den_merge(kt):
                        pt = pts.pop(kt)
                        if kt == 0:
                            nc.vector.tensor_copy(dacc[:], pt[:])
                        else:
                            o = qoff(kt)
                            nc.vector.tensor_tensor(
                                dacc[:, o:], dacc[:, o:], pt[:, o:],
                                mybir.AluOpType.add)

                    qk_exp(0)
                    qk_exp(1)
                    for kt in range(2, nkt):
                        k = kt - 2
                        pv(k)
                        den_merge(k)
                        qk_exp(kt)
                    pv(nkt - 2)
                    den_merge(nkt - 2)
                    pv(nkt - 1)
                    den_merge(nkt - 1)

                    pd = ps.tile([128, TC], F32, tag="s", bufs=2, name="pdn")
                    nc.tensor.matmul(pd[:], ones_sb[:], dacc[:],
                                     start=True, stop=True)
                    rc = sb.tile([128, TC], F32, tag="rc", bufs=1, name="rc")
                    nc.vector.reciprocal_approx_fast(rc[:], pd[:])
                    nc.vector.tensor_tensor(
                        attnb[:, h * TC:(h + 1) * TC], pa[:], rc[:],
                        mybir.AluOpType.mult)
                    if h == 3 or h == 7:
                        ccos.append(gather_piece(attnb, h - 3))
                return ccos

            def gather_piece(attnb, h0):
                """AllGather heads [h0, h0+4) of attnb -> cco [4*4*128, TC]."""
                cci = dr.tile([4 * 128, TC], BF16, tag="cci", bufs=8,
                              name="cci")
                cco = dr.tile([TP * 4 * 128, TC], BF16, tag="cco", bufs=8,
                              name="cco")
                nc.sync.dma_start(
                    cci.rearrange("(h p) t -> p h t", p=128),
                    attnb.rearrange("p (h t) -> p h t", h=HL)[:, h0:h0 + 4])
                nc.gpsimd.collective_compute(
                    "AllGather", mybir.AluOpType.bypass,
                    replica_groups=GROUPS, ins=[cci[:]], outs=[cco[:]])
                return cco

            def load_af(cco, floor_ms):
                """cco piece -> SBUF [128, 16*TC] (g = r*4 + local head).

                floor_ms is a scheduler hint: the tile list-scheduler's sim
                has no model of AllGather latency, so without a floor it
                schedules af-consuming matmuls right after the collective
                trigger and the PE stalls on the real collective."""
                af = sb.tile([128, 16 * TC], BF16, tag="af", bufs=2, name="af")
                with tc.tile_wait_until(floor_ms):
                    nc.sync.dma_start(
                        af.rearrange("p (g t) -> p g t", g=16),
                        cco.rearrange("(g p) t -> p g t", p=128))
                return af

            def af_slice(afs, dt):
                """af slice for global d-tile dt (wo row-block r*8+hh)."""
                r, hh = dt // 8, dt % 8
                af = afs[hh // 4]
                slot = r * 4 + (hh % 4)
                return af[:, slot * TC:(slot + 1) * TC]

            def outproj(c, ccos, floors):
                # NOTE: outproj uses dedicated "wo"/"po" pools: its
                # execution is collective-gated, and same-tag pool buffers
                # recycle in allocation order -- sharing "wb"/"pj" with a
                # later proj() would WAR-chain that proj behind the
                # collective wait.
                t0 = c * TC
                afs = [load_af(ccos[0], floors[0]), load_af(ccos[1], floors[1])]
                for o in range(OC // 128):
                    wb = sb.tile([128, PANW], BF16, tag="wo", bufs=2,
                                 name="wo")
                    nc.sync.dma_start(
                        wb[:], w_d[:, (12 + o) * PANW:(13 + o) * PANW])
                    po = ps.tile([128, TC], F32, tag="po", bufs=2, name="po")
                    for i, dt in enumerate(range(KT)):
                        nc.tensor.matmul(
                            po[:], wb[:, dt * 128:(dt + 1) * 128],
                            af_slice(afs, dt),
                            start=(i == 0), stop=(i == KT - 1))
                    ot = sb.tile([128, TC], F32, tag="ot", bufs=2, name="ot")
                    nc.vector.tensor_copy(ot[:], po[:])
                    nc.sync.dma_start(
                        out_d[o * 128:(o + 1) * 128, t0:t0 + TC], ot[:])

            # ---- schedule ----
            # Chunk-interleaved: proj(c) -> attn(c) so collective pieces
            # trigger early and the serialized AllGather chain hides
            # behind remaining matmul work. Outprojs are PE filler once
            # their afs are (estimated) complete.
            load_xt(0, engs=[nc.sync, nc.scalar, nc.gpsimd, nc.scalar])
            nc.scalar.dma_start(cos_sb[:], cos_d[:])
            nc.scalar.dma_start(sin_sb[:], sin_d[:])
            nc.gpsimd.dma_start(mask_sb[:], mask_d[:])
            nc.gpsimd.dma_start(ones_sb[:], ones_d[:])

            spread = [nc.scalar, nc.gpsimd, nc.sync, nc.scalar]
            qtbs = {}
            gath = {}
            qtbs[0] = proj(0)
            load_xt(1, engs=spread)
            gath[0] = attention(0, qtbs[0])
            qtbs[1] = proj(1)
            load_xt(2, engs=spread)
            gath[1] = attention(1, qtbs[1])
            outproj(0, gath[0], (0.21, 0.24))
            qtbs[2] = proj(2)
            load_xt(3, engs=spread)
            gath[2] = attention(2, qtbs[2])
            outproj(1, gath[1], (0.28, 0.31))
            qtbs[3] = proj(3)
            gath[3] = attention(3, qtbs[3])
            outproj(2, gath[2], (0.42, 0.45))
            outproj(3, gath[3], (0.55, 0.58))

    nc.compile()
    return nc


def _get_nc():
    if "nc" not in _BUILT:
        _BUILT["nc"] = _build()
    return _BUILT["nc"]


def _to_bf16(a):
    return np.ascontiguousarray(a.astype(ml_dtypes.bfloat16))


def _panelize(w):
    """[H, C] fp32 -> [128, KT*C] bf16 in [p, kt, c] layout."""
    c = w.shape[1]
    return _to_bf16(
        w.reshape(KT, 128, c).transpose(1, 0, 2).reshape(128, KT * c))


def kernel(hidden_states, cos, sin, wq, wk, wv, wo):
    global LAST_RESULT
    nc = _get_nc()

    hidden_states = np.asarray(hidden_states, dtype=np.float32)
    cos = np.asarray(cos, dtype=np.float32)
    sin = np.asarray(sin, dtype=np.float32)
    wq = np.asarray(wq, dtype=np.float32)
    wk = np.asarray(wk, dtype=np.float32)
    wv = np.asarray(wv, dtype=np.float32)
    wo = np.asarray(wo, dtype=np.float32)

    # host-side shard prep: X^T as [128, kt*S] bf16
    xts = []
    for b in range(B):
        xt = hidden_states[b].T.reshape(KT, 128, S).transpose(1, 0, 2)
        xts.append(_to_bf16(xt.reshape(128, KT * S)))
    cts = [np.ascontiguousarray(cos[b].T) for b in range(B)]
    sin_eff = []
    for b in range(B):
        st = np.ascontiguousarray(sin[b].T)
        se = st.copy()
        se[0:64, :] *= -1.0
        sin_eff.append(se)

    maskb = np.triu(np.ones((128, 128))).astype(ml_dtypes.bfloat16)
    onesb = np.ones((128, 128), dtype=ml_dtypes.bfloat16)

    # weight panels per tp rank: 8 wq, 2 wk, 1 double wv, 8 wo
    w_alls = []
    for r in range(TP):
        panels = []
        for h in range(HL):
            panels.append(_panelize(wq[:, r * QCOLS + h * 128:
                                       r * QCOLS + (h + 1) * 128]))
        for kv in range(KVL):
            panels.append(_panelize(wk[:, r * (KVL * 128) + kv * 128:
                                       r * (KVL * 128) + (kv + 1) * 128]))
        panels.append(_panelize(wv[:, r * (KVL * 128):
                                   (r + 1) * (KVL * 128)]))
        for o in range(OC // 128):
            panels.append(_panelize(wo[:, r * OC + o * 128:
                                       r * OC + (o + 1) * 128]))
        w_alls.append(np.concatenate(panels, axis=1))

    in_maps = []
    for core in range(N_CORES):
        b, r = core // TP, core % TP
        in_maps.append({
            "xt": xts[b],
            "w_all": w_alls[r],
            "cos_t": cts[b],
            "sin_t": sin_eff[b],
            "maskb": maskb,
            "onesb": onesb,
        })

    res = run_bass_kernel_spmd(nc, in_maps, core_ids=list(range(N_CORES)))
    LAST_RESULT = res

    out = np.empty((B, S, H), dtype=np.float32)
    for core in range(N_CORES):
        b, r = core // TP, core % TP
        out[b, :, r * OC:(r + 1) * OC] = res.results[core]["out_t"].T
    return out


# revision 23
# speedup vs baseline: 1.0337x; 1.0337x over previous
"""Llama GQA attention (B=2, S=2048, H=4096, 32 q heads / 8 kv heads, HD=128)
on 8 Trainium2 NeuronCores.

Sharding: DP=2 over batch x TP=4 over heads.
  core c: batch b = c // 4, tp rank r = c % 4
  - owns q heads [8r, 8r+8), kv heads [2r, 2r+2)
  - AllGather (bf16) of attention outputs within each 4-core batch group,
    split in two 4-head pieces triggered mid-attention
  - output projection over the full 4096 attn features for output columns
    [1024r, 1024(r+1)) -> disjoint outputs, host concatenates.

All inputs are pre-cast/pre-transposed to the on-chip layout on the HOST
(bf16 weights/X^T), so the device does zero dtype conversion and reads
every operand exactly once per use:
  xt:  [128, kt*S + t]   X^T with hidden dim split into 32 k-tiles
  w:   [128, panel*4096] weight panels (8 wq, 2 wk, 1 double-width wv,
                          8 wo), each [128 k-part, kt*cols + c]
On-chip layout is "transposed" ([feature, token]) for Q/K; V is computed
DIRECTLY in [token, dim] layout (lhsT = X^T tile stationary, wv panel
moving, N=256 covering both kv heads) -- no PE transposes needed.
Causal masking: per q-chunk c only k-tiles 0..4c+3 are touched; the 4
diagonal tiles compute only the surviving q-range (free = 512-128j) plus
one static [128,128] triangular mask on the leading square (GpSimd).
Softmax skips max-subtraction (scores are O(8), exp fits bf16). The
denominator merges each adjacent pair of exp tiles with one in-place
DVE bf16 add and runs a single ones-matmul per pair on the PE (half
the denominator matmuls); the reciprocal uses the fast DVE
approximation (18 bits).

Schedule (v2): chunk-interleaved -- proj(c) then IMMEDIATELY attn(c)
(possible because attn(c) only needs K/V chunks 0..c), so the first
AllGather piece triggers ~120us into the kernel instead of ~480us.
The 8 AllGather pieces serialize on the collective stream (~25-45us
each, ~90us for the cold first one); triggering them early hides the
entire ~350us chain behind remaining proj/attn/outproj matmul work
instead of exposing its tail. Out-projections are placed as PE filler:
op0 after attn1, op1 after attn2, op2+op3 after attn3. The af loads
carry tile_wait_until floors approximating real collective completion
so the list scheduler (whose sim has no collective-latency model) does
not hoist af-consuming matmuls ahead of available attention work.
Startup DMAs are posted xt-first across four engine queues so the
first matmul issues ~12us in instead of ~30us.
"""

import sys

for _p in ("/opt/trn_rl_repo",):
    if _p not in sys.path:
        sys.path.append(_p)

import numpy as np
import ml_dtypes

import concourse.bacc as bacc
import concourse.mybir as mybir
import concourse.tile as tile
from concourse.bass_utils import run_bass_kernel_spmd

F32 = mybir.dt.float32
BF16 = mybir.dt.bfloat16

B, S, H = 2, 2048, 4096
NH, NKV, HD = 32, 8, 128
N_CORES = 8
TP = 4
GROUPS = [[0, 1, 2, 3], [4, 5, 6, 7]]

HL = NH // TP          # 8 local q heads
KVL = NKV // TP        # 2 local kv heads
QCOLS = HL * HD        # 1024 local q cols
OC = H // TP           # 1024 local out cols

TC = 512               # token chunk (= one attention q-block)
NCHUNK = S // TC       # 4
KT = H // 128          # 32 contraction tiles
SCALE = float(HD ** -0.5)
NPANEL = 20            # 8 wq + 2 wk + 2 (double wv) + 8 wo
PANW = KT * 128        # 4096 free cols per standard weight panel

LAST_RESULT = None
_BUILT = {}


def _build():
    nc = bacc.Bacc("TRN2", debug=False, num_devices=N_CORES)

    xt_d = nc.dram_tensor("xt", [128, KT * S], BF16, kind="ExternalInput").ap()
    w_d = nc.dram_tensor("w_all", [128, NPANEL * PANW], BF16,
                         kind="ExternalInput").ap()
    cos_d = nc.dram_tensor("cos_t", [HD, S], F32, kind="ExternalInput").ap()
    sin_d = nc.dram_tensor("sin_t", [HD, S], F32, kind="ExternalInput").ap()
    mask_d = nc.dram_tensor("maskb", [128, 128], BF16, kind="ExternalInput").ap()
    ones_d = nc.dram_tensor("onesb", [128, 128], BF16, kind="ExternalInput").ap()
    out_d = nc.dram_tensor("out_t", [OC, S], F32, kind="ExternalOutput").ap()

    with tile.TileContext(nc) as tc:
        with tc.tile_pool(name="sb", bufs=1) as sb, \
             tc.tile_pool(name="ps", bufs=1, space="PSUM") as ps, \
             tc.tile_pool(name="dr", bufs=1, space="DRAM") as dr:

            # ---- persistent tiles ----
            cos_sb = sb.tile([HD, S], F32)
            sin_sb = sb.tile([HD, S], F32)
            mask_sb = sb.tile([128, 128], BF16)
            ones_sb = sb.tile([128, 128], BF16)
            ktb = sb.tile([128, KVL * S], BF16)            # roped K^T [d, kv*S+t]
            vb = sb.tile([128, (S // 128) * KVL * 128], BF16)  # V [t, tt*256+d]

            _XTB = {}

            def load_xt(c, engs=None):
                """X^T chunk c -> four SBUF quarter-tiles [128, 8*TC].

                Quarters with bufs=7 let chunk c+1's loads start while
                chunk c is still being consumed (a monolithic bufs=1 tile
                made the load wait for proj(c)'s LAST matmul, stalling the
                in-order PE queue ~20us per chunk)."""
                engs = engs or [nc.scalar, nc.gpsimd, nc.scalar, nc.gpsimd]
                quarters = []
                for qf in range(4):
                    xth = sb.tile([128, (KT // 4) * TC], BF16, tag="xtb",
                                  bufs=7, name=f"xtb{c}_{qf}")
                    k0 = qf * (KT // 4)
                    engs[qf].dma_start(
                        xth.rearrange("p (kt t) -> p kt t", t=TC),
                        xt_d.rearrange("p (kt t) -> p kt t", t=S)
                        [:, k0:k0 + KT // 4, c * TC:(c + 1) * TC])
                    quarters.append(xth)
                _XTB[c] = quarters

            def xt_slice(c, kt, lo, hi):
                """X^T [128, lo:hi] slice of k-tile kt for chunk c."""
                xth = _XTB[c][kt // 8]
                base = (kt % 8) * TC
                return xth[:, base + lo:base + hi]

            def get_panel(idx, eng=None, tag="wb", split=2):
                wb = sb.tile([128, PANW], BF16, tag=tag, bufs=2, name=tag)
                step = PANW // split
                for si in range(split):
                    (eng or nc.sync).dma_start(
                        wb[:, si * step:(si + 1) * step],
                        w_d[:, idx * PANW + si * step:
                            idx * PANW + (si + 1) * step])
                return wb

            def get_wv2(eng=None):
                """Double-width V panel [128, kt*256 + d] at slot 10."""
                wb = sb.tile([128, 2 * PANW], BF16, tag="wv2", bufs=1,
                             name="wv2")
                (eng or nc.sync).dma_start(
                    wb[:], w_d[:, 10 * PANW:12 * PANW])
                return wb

            def rope(dst, pq, t0):
                """dst (bf16 [128, TC]) = rope of pq (fp32 PSUM [128, TC])."""
                qf = sb.tile([128, TC], F32, tag="qf", bufs=1)
                nc.scalar.copy(qf[:], pq[:])
                qs = sb.tile([128, TC], F32, tag="qs", bufs=1)
                nc.sync.dma_start(qs[0:64, :], qf[64:128, :])
                nc.sync.dma_start(qs[64:128, :], qf[0:64, :])
                nc.vector.tensor_tensor(
                    qf[:], qf[:], cos_sb[:, t0:t0 + TC], mybir.AluOpType.mult)
                nc.vector.tensor_tensor(
                    qs[:], qs[:], sin_sb[:, t0:t0 + TC], mybir.AluOpType.mult)
                nc.vector.tensor_tensor(dst, qf[:], qs[:], mybir.AluOpType.add)

            def proj(c):
                t0 = c * TC
                qtb = sb.tile([128, HL * TC], BF16, tag="qt", bufs=2,
                              name="qtb")
                wv = get_wv2(nc.gpsimd)   # post early on an idle DMA queue
                if c + 1 < NCHUNK:
                    load_xt(c + 1)        # posted now; fires as WARs clear
                for h in range(HL):
                    wb = get_panel(h)
                    pq = ps.tile([128, TC], F32, tag="pj", bufs=2, name="pq")
                    for kt in range(KT):
                        nc.tensor.matmul(
                            pq[:], wb[:, kt * 128:(kt + 1) * 128],
                            xt_slice(c, kt, 0, TC),
                            start=(kt == 0), stop=(kt == KT - 1))
                    rope(qtb[:, h * TC:(h + 1) * TC], pq, t0)
                for kv in range(KVL):
                    wb = get_panel(8 + kv)
                    pk = ps.tile([128, TC], F32, tag="pj", bufs=2, name="pk")
                    for kt in range(KT):
                        nc.tensor.matmul(
                            pk[:], wb[:, kt * 128:(kt + 1) * 128],
                            xt_slice(c, kt, 0, TC),
                            start=(kt == 0), stop=(kt == KT - 1))
                    rope(ktb[:, kv * S + t0:kv * S + t0 + TC], pk, t0)
                # direct V: out[t, d] accumulated with X^T tile stationary,
                # wv panel moving (N=256 covers both kv heads). Two token
                # tiles share one PSUM bank -- SEQUENTIAL groups only:
                # start_tensor_calc marks a whole 2KB zero-region, so
                # interleaving two live groups in one bank corrupts the
                # earlier group's accumulation.
                for tp_ in range(2):
                    pvd = ps.tile([128, TC], F32, tag="pj", bufs=2, name="pvd")
                    for half in range(2):
                        tb = 2 * tp_ + half
                        for kt in range(KT):
                            nc.tensor.matmul(
                                pvd[:, half * 256:(half + 1) * 256],
                                xt_slice(c, kt, tb * 128, tb * 128 + 128),
                                wv[:, kt * 256:(kt + 1) * 256],
                                start=(kt == 0), stop=(kt == KT - 1))
                    vt0 = 4 * c + 2 * tp_
                    nc.scalar.copy(
                        vb[:, vt0 * 256:(vt0 + 2) * 256], pvd[:])
                return qtb

            def attention(c, qtb):
                nkt = 4 * c + 4
                attnb = sb.tile([128, HL * TC], BF16, tag="attn", bufs=1,
                                name="attnb")
                ccos = []
                for h in range(HL):
                    kv = h // (HL // KVL)
                    qsl = qtb[:, h * TC:(h + 1) * TC]
                    pa = ps.tile([128, TC], F32, tag="pa", bufs=2, name="pa")
                    # denominator accumulator (bf16 SBUF): exp tiles are
                    # summed here by DVE adds; ONE ones-matmul per head at
                    # the end replaces per-pair denominator matmuls on PE.
                    dacc = sb.tile([128, TC], BF16, tag="dacc", bufs=1,
                                   name="dacc")
                    pts = {}

                    def qoff(kt):
                        j = kt - 4 * c
                        return 128 * j if j >= 0 else 0

                    def qk_exp(kt):
                        o = qoff(kt)
                        sps = ps.tile([128, TC], F32, tag="s", bufs=2,
                                      name="sps")
                        nc.tensor.matmul(
                            sps[:, o:],
                            ktb[:, kv * S + kt * 128:kv * S + (kt + 1) * 128],
                            qsl[:, o:], start=True, stop=True)
                        pt = sb.tile([128, TC], BF16, tag="pt", bufs=6,
                                     name="pt")
                        nc.scalar.activation(
                            pt[:, o:], sps[:, o:],
                            mybir.ActivationFunctionType.Exp, scale=SCALE)
                        if kt - 4 * c >= 0:
                            nc.gpsimd.tensor_tensor(
                                pt[:, o:o + 128], pt[:, o:o + 128], mask_sb[:],
                                mybir.AluOpType.mult)
                        pts[kt] = pt

                    def pv(kt):
                        o = qoff(kt)
                        nc.tensor.matmul(
                            pa[:, o:],
                            vb[:, kt * (KVL * 128) + kv * 128:
                               kt * (KVL * 128) + (kv + 1) * 128],
                            pts[kt][:, o:],
                            start=(kt == 0), stop=(kt == nkt - 1))

                    def den_merge(kt):
                        pt = pts.pop(kt)
                        if kt == 0:
                            nc.vector.tensor_copy(dacc[:], pt[:])
                        else:
                            o = qoff(kt)
                            nc.vector.tensor_tensor(
                                dacc[:, o:], dacc[:, o:], pt[:, o:],
                                mybir.AluOpType.add)

                    qk_exp(0)
                    qk_exp(1)
                    for kt in range(2, nkt):
                        k = kt - 2
                        pv(k)
                        den_merge(k)
                        qk_exp(kt)
                    pv(nkt - 2)
                    den_merge(nkt - 2)
                    pv(nkt - 1)
                    den_merge(nkt - 1)

                    pd = ps.tile([128, TC], F32, tag="s", bufs=2, name="pdn")
                    nc.tensor.matmul(pd[:], ones_sb[:], dacc[:],
                                     start=True, stop=True)
                    rc = sb.tile([128, TC], F32, tag="rc", bufs=1, name="rc")
                    nc.vector.reciprocal_approx_fast(rc[:], pd[:])
                    nc.vector.tensor_tensor(
                        attnb[:, h * TC:(h + 1) * TC], pa[:], rc[:],
                        mybir.AluOpType.mult)
                    if h == 3 or h == 7:
                        ccos.append(gather_piece(attnb, h - 3))
                return ccos

            def gather_piece(attnb, h0):
                """AllGather heads [h0, h0+4) of attnb -> cco [4*4*128, TC]."""
                cci = dr.tile([4 * 128, TC], BF16, tag="cci", bufs=8,
                              name="cci")
                cco = dr.tile([TP * 4 * 128, TC], BF16, tag="cco", bufs=8,
                              name="cco")
                nc.sync.dma_start(
                    cci.rearrange("(h p) t -> p h t", p=128),
                    attnb.rearrange("p (h t) -> p h t", h=HL)[:, h0:h0 + 4])
                nc.gpsimd.collective_compute(
                    "AllGather", mybir.AluOpType.bypass,
                    replica_groups=GROUPS, ins=[cci[:]], outs=[cco[:]])
                return cco

            def load_af(cco, floor_ms):
                """cco piece -> SBUF [128, 16*TC] (g = r*4 + local head).

                floor_ms is a scheduler hint: the tile list-scheduler's sim
                has no model of AllGather latency, so without a floor it
                schedules af-consuming matmuls right after the collective
                trigger and the PE stalls on the real collective."""
                af = sb.tile([128, 16 * TC], BF16, tag="af", bufs=2, name="af")
                with tc.tile_wait_until(floor_ms):
                    nc.sync.dma_start(
                        af.rearrange("p (g t) -> p g t", g=16),
                        cco.rearrange("(g p) t -> p g t", p=128))
                return af

            def af_slice(afs, dt):
                """af slice for global d-tile dt (wo row-block r*8+hh)."""
                r, hh = dt // 8, dt % 8
                af = afs[hh // 4]
                slot = r * 4 + (hh % 4)
                return af[:, slot * TC:(slot + 1) * TC]

            # dts whose af slice comes from piece 0 (hh%8 < 4), then piece 1:
            # accumulating piece-0 rows first lets each panel start before
            # the second collective piece lands.
            DT_ORDER = ([dt for dt in range(KT) if dt % 8 < 4] +
                        [dt for dt in range(KT) if dt % 8 >= 4])

            def outproj(c, ccos, floors, wide=False):
                # NOTE: outproj uses dedicated "wo"/"po" pools: its
                # execution is collective-gated, and same-tag pool buffers
                # recycle in allocation order -- sharing "wb"/"pj" with a
                # later proj() would WAR-chain that proj behind the
                # collective wait. The LAST outproj (wide=True) instead
                # round-robins all four PSUM tags (everything else is done
                # by then) so all 8 panels get distinct banks and the
                # piece-0 halves all run while the last AllGather flies.
                t0 = c * TC
                afs = [load_af(ccos[0], floors[0]), load_af(ccos[1], floors[1])]
                for o in range(OC // 128):
                    wb = get_panel(12 + o, tag="wo")
                    ptag = ["po", "pa", "s", "pj"][o % 4] if wide else "po"
                    po = ps.tile([128, TC], F32, tag=ptag, bufs=2, name="po")
                    for i, dt in enumerate(DT_ORDER):
                        nc.tensor.matmul(
                            po[:], wb[:, dt * 128:(dt + 1) * 128],
                            af_slice(afs, dt),
                            start=(i == 0), stop=(i == KT - 1))
                    ot = sb.tile([128, TC], F32, tag="ot", bufs=1, name="ot")
                    nc.vector.tensor_copy(ot[:], po[:])
                    nc.sync.dma_start(
                        out_d[o * 128:(o + 1) * 128, t0:t0 + TC], ot[:])

            # ---- schedule ----
            # Chunk-interleaved: proj(c) -> attn(c) so collective pieces
            # trigger early and the serialized AllGather chain hides
            # behind remaining matmul work. Outprojs are PE filler once
            # their afs are (estimated) complete.
            # Startup: sync queue is reserved for weight panels (first MM
            # needs panel 0); X^T goes on scalar+gpsimd queues; cos/sin
            # are not needed until the first rope's DVE ops (~20us) and
            # mask/ones not until attn0 (~120us), so they queue last.
            load_xt(0)
            nc.scalar.dma_start(cos_sb[:], cos_d[:])
            nc.scalar.dma_start(sin_sb[:], sin_d[:])
            nc.gpsimd.dma_start(mask_sb[:], mask_d[:])
            nc.gpsimd.dma_start(ones_sb[:], ones_d[:])
            # Dummy tiny AllGather to absorb the cold-start cost of the
            # collective stream during proj(0).
            dgi = dr.tile([128, 128], BF16, tag="dgi", bufs=1, name="dgi")
            dgo = dr.tile([TP * 128, 128], BF16, tag="dgo", bufs=1,
                          name="dgo")
            nc.gpsimd.dma_start(dgi[:], mask_d[:])
            nc.gpsimd.collective_compute(
                "AllGather", mybir.AluOpType.bypass,
                replica_groups=GROUPS, ins=[dgi[:]], outs=[dgo[:]])

            qtbs = {}
            gath = {}
            qtbs[0] = proj(0)
            gath[0] = attention(0, qtbs[0])
            qtbs[1] = proj(1)
            gath[1] = attention(1, qtbs[1])
            outproj(0, gath[0], (0.23, 0.26))
            qtbs[2] = proj(2)
            gath[2] = attention(2, qtbs[2])
            outproj(1, gath[1], (0.31, 0.35))
            qtbs[3] = proj(3)
            gath[3] = attention(3, qtbs[3])
            outproj(2, gath[2], (0.44, 0.47))
            outproj(3, gath[3], (0.60, 0.64), wide=True)

    nc.compile()
    return nc


def _get_nc():
    if "nc" not in _BUILT:
        _BUILT["nc"] = _build()
    return _BUILT["nc"]


def _to_bf16(a):
    return np.ascontiguousarray(a.astype(ml_dtypes.bfloat16))


def _panelize(w):
    """[H, C] fp32 -> [128, KT*C] bf16 in [p, kt, c] layout."""
    c = w.shape[1]
    return _to_bf16(
        w.reshape(KT, 128, c).transpose(1, 0, 2).reshape(128, KT * c))


def kernel(hidden_states, cos, sin, wq, wk, wv, wo):
    global LAST_RESULT
    nc = _get_nc()

    hidden_states = np.asarray(hidden_states, dtype=np.float32)
    cos = np.asarray(cos, dtype=np.float32)
    sin = np.asarray(sin, dtype=np.float32)
    wq = np.asarray(wq, dtype=np.float32)
    wk = np.asarray(wk, dtype=np.float32)
    wv = np.asarray(wv, dtype=np.float32)
    wo = np.asarray(wo, dtype=np.float32)

    # host-side shard prep: X^T as [128, kt*S] bf16
    xts = []
    for b in range(B):
        xt = hidden_states[b].T.reshape(KT, 128, S).transpose(1, 0, 2)
        xts.append(_to_bf16(xt.reshape(128, KT * S)))
    cts = [np.ascontiguousarray(cos[b].T) for b in range(B)]
    sin_eff = []
    for b in range(B):
        st = np.ascontiguousarray(sin[b].T)
        se = st.copy()
        se[0:64, :] *= -1.0
        sin_eff.append(se)

    maskb = np.triu(np.ones((128, 128))).astype(ml_dtypes.bfloat16)
    onesb = np.ones((128, 128), dtype=ml_dtypes.bfloat16)

    # weight panels per tp rank: 8 wq, 2 wk, 1 double wv, 8 wo
    w_alls = []
    for r in range(TP):
        panels = []
        for h in range(HL):
            panels.append(_panelize(wq[:, r * QCOLS + h * 128:
                                       r * QCOLS + (h + 1) * 128]))
        for kv in range(KVL):
            panels.append(_panelize(wk[:, r * (KVL * 128) + kv * 128:
                                       r * (KVL * 128) + (kv + 1) * 128]))
        panels.append(_panelize(wv[:, r * (KVL * 128):
                                   (r + 1) * (KVL * 128)]))
        for o in range(OC // 128):
            panels.append(_panelize(wo[:, r * OC + o * 128:
                                       r * OC + (o + 1) * 128]))
        w_alls.append(np.concatenate(panels, axis=1))

    in_maps = []
    for core in range(N_CORES):
        b, r = core // TP, core % TP
        in_maps.append({
            "xt": xts[b],
            "w_all": w_alls[r],
            "cos_t": cts[b],
            "sin_t": sin_eff[b],
            "maskb": maskb,
            "onesb": onesb,
        })

    res = run_bass_kernel_spmd(nc, in_maps, core_ids=list(range(N_CORES)))
    LAST_RESULT = res

    out = np.empty((B, S, H), dtype=np.float32)
    for core in range(N_CORES):
        b, r = core // TP, core % TP
        out[b, :, r * OC:(r + 1) * OC] = res.results[core]["out_t"].T
    return out


# revision 24
# speedup vs baseline: 1.0744x; 1.0394x over previous
"""Llama GQA attention (B=2, S=2048, H=4096, 32 q heads / 8 kv heads, HD=128)
on 8 Trainium2 NeuronCores.

Sharding: DP=2 over batch x TP=4 over heads.
  core c: batch b = c // 4, tp rank r = c % 4
  - owns q heads [8r, 8r+8), kv heads [2r, 2r+2)
  - AllGather (bf16) of attention outputs within each 4-core batch group,
    split in two 4-head pieces triggered mid-attention
  - output projection over the full 4096 attn features for output columns
    [1024r, 1024(r+1)) -> disjoint outputs, host concatenates.

All inputs are pre-cast/pre-transposed to the on-chip layout on the HOST
(bf16 weights/X^T), so the device does zero dtype conversion and reads
every operand exactly once per use:
  xt:  [128, kt*S + t]   X^T with hidden dim split into 32 k-tiles
  w:   [128, panel*4096] weight panels (8 wq, 2 wk, 1 double-width wv,
                          8 wo), each [128 k-part, kt*cols + c]
On-chip layout is "transposed" ([feature, token]) for Q/K; V is computed
DIRECTLY in [token, dim] layout (lhsT = X^T tile stationary, wv panel
moving, N=256 covering both kv heads) -- no PE transposes needed.
Causal masking: per q-chunk c only k-tiles 0..4c+3 are touched; the 4
diagonal tiles compute only the surviving q-range (free = 512-128j) plus
one static [128,128] triangular mask on the leading square (GpSimd).
Softmax skips max-subtraction (scores are O(8), exp fits bf16). The
denominator merges each adjacent pair of exp tiles with one in-place
DVE bf16 add and runs a single ones-matmul per pair on the PE (half
the denominator matmuls); the reciprocal uses the fast DVE
approximation (18 bits).

Schedule (v2): chunk-interleaved -- proj(c) then IMMEDIATELY attn(c)
(possible because attn(c) only needs K/V chunks 0..c), so the first
AllGather piece triggers ~120us into the kernel instead of ~480us.
The 8 AllGather pieces serialize on the collective stream (~25-45us
each, ~90us for the cold first one); triggering them early hides the
entire ~350us chain behind remaining proj/attn/outproj matmul work
instead of exposing its tail. Out-projections are placed as PE filler:
op0 after attn1, op1 after attn2, op2+op3 after attn3. The af loads
carry tile_wait_until floors approximating real collective completion
so the list scheduler (whose sim has no collective-latency model) does
not hoist af-consuming matmuls ahead of available attention work.
Startup DMAs are posted xt-first across four engine queues so the
first matmul issues ~12us in instead of ~30us.
"""

import sys

for _p in ("/opt/trn_rl_repo",):
    if _p not in sys.path:
        sys.path.append(_p)

import numpy as np
import ml_dtypes

import concourse.bacc as bacc
import concourse.mybir as mybir
import concourse.tile as tile
from concourse.bass_utils import run_bass_kernel_spmd

F32 = mybir.dt.float32
BF16 = mybir.dt.bfloat16

B, S, H = 2, 2048, 4096
NH, NKV, HD = 32, 8, 128
N_CORES = 8
TP = 4
GROUPS = [[0, 1, 2, 3], [4, 5, 6, 7]]

HL = NH // TP          # 8 local q heads
KVL = NKV // TP        # 2 local kv heads
QCOLS = HL * HD        # 1024 local q cols
OC = H // TP           # 1024 local out cols

TC = 512               # token chunk (= one attention q-block)
NCHUNK = S // TC       # 4
KT = H // 128          # 32 contraction tiles
SCALE = float(HD ** -0.5)
NPANEL = 20            # 8 wq + 2 wk + 2 (double wv) + 8 wo
PANW = KT * 128        # 4096 free cols per standard weight panel

LAST_RESULT = None
_BUILT = {}


def _build():
    nc = bacc.Bacc("TRN2", debug=False, num_devices=N_CORES)

    xt_d = nc.dram_tensor("xt", [128, KT * S], BF16, kind="ExternalInput").ap()
    w_d = nc.dram_tensor("w_all", [128, NPANEL * PANW], BF16,
                         kind="ExternalInput").ap()
    cos_d = nc.dram_tensor("cos_t", [HD, S], F32, kind="ExternalInput").ap()
    sin_d = nc.dram_tensor("sin_t", [HD, S], F32, kind="ExternalInput").ap()
    mask_d = nc.dram_tensor("maskb", [128, 128], BF16, kind="ExternalInput").ap()
    ones_d = nc.dram_tensor("onesb", [128, 128], BF16, kind="ExternalInput").ap()
    out_d = nc.dram_tensor("out_t", [OC, S], F32, kind="ExternalOutput").ap()

    with tile.TileContext(nc) as tc:
        with tc.tile_pool(name="sb", bufs=1) as sb, \
             tc.tile_pool(name="ps", bufs=1, space="PSUM") as ps, \
             tc.tile_pool(name="dr", bufs=1, space="DRAM") as dr:

            # ---- persistent tiles ----
            cos_sb = sb.tile([HD, S], F32)
            sin_sb = sb.tile([HD, S], F32)
            mask_sb = sb.tile([128, 128], BF16)
            ones_sb = sb.tile([128, 128], BF16)
            ktb = sb.tile([128, KVL * S], BF16)            # roped K^T [d, kv*S+t]
            vb = sb.tile([128, (S // 128) * KVL * 128], BF16)  # V [t, tt*256+d]

            _XTB = {}

            def load_xt(c, engs=None):
                """X^T chunk c -> four SBUF quarter-tiles [128, 8*TC].

                Quarters with bufs=7 let chunk c+1's loads start while
                chunk c is still being consumed (a monolithic bufs=1 tile
                made the load wait for proj(c)'s LAST matmul, stalling the
                in-order PE queue ~20us per chunk)."""
                engs = engs or [nc.scalar, nc.gpsimd, nc.scalar, nc.gpsimd]
                quarters = []
                for qf in range(4):
                    xth = sb.tile([128, (KT // 4) * TC], BF16, tag="xtb",
                                  bufs=7, name=f"xtb{c}_{qf}")
                    k0 = qf * (KT // 4)
                    engs[qf].dma_start(
                        xth.rearrange("p (kt t) -> p kt t", t=TC),
                        xt_d.rearrange("p (kt t) -> p kt t", t=S)
                        [:, k0:k0 + KT // 4, c * TC:(c + 1) * TC])
                    quarters.append(xth)
                _XTB[c] = quarters

            def xt_slice(c, kt, lo, hi):
                """X^T [128, lo:hi] slice of k-tile kt for chunk c."""
                xth = _XTB[c][kt // 8]
                base = (kt % 8) * TC
                return xth[:, base + lo:base + hi]

            def get_panel(idx, eng=None, tag="wb", split=2):
                wb = sb.tile([128, PANW], BF16, tag=tag, bufs=2, name=tag)
                step = PANW // split
                for si in range(split):
                    (eng or nc.sync).dma_start(
                        wb[:, si * step:(si + 1) * step],
                        w_d[:, idx * PANW + si * step:
                            idx * PANW + (si + 1) * step])
                return wb

            def get_wv2(eng=None):
                """Double-width V panel [128, kt*256 + d] at slot 10."""
                wb = sb.tile([128, 2 * PANW], BF16, tag="wv2", bufs=1,
                             name="wv2")
                (eng or nc.sync).dma_start(
                    wb[:], w_d[:, 10 * PANW:12 * PANW])
                return wb

            def rope(dst, pq, t0):
                """dst (bf16 [128, TC]) = rope of pq (fp32 PSUM [128, TC])."""
                qf = sb.tile([128, TC], F32, tag="qf", bufs=1)
                nc.scalar.copy(qf[:], pq[:])
                qs = sb.tile([128, TC], F32, tag="qs", bufs=1)
                nc.sync.dma_start(qs[0:64, :], qf[64:128, :])
                nc.sync.dma_start(qs[64:128, :], qf[0:64, :])
                nc.vector.tensor_tensor(
                    qf[:], qf[:], cos_sb[:, t0:t0 + TC], mybir.AluOpType.mult)
                nc.vector.tensor_tensor(
                    qs[:], qs[:], sin_sb[:, t0:t0 + TC], mybir.AluOpType.mult)
                nc.vector.tensor_tensor(dst, qf[:], qs[:], mybir.AluOpType.add)

            def proj_panel(c, idx, nm):
                wb = get_panel(idx)
                pp = ps.tile([128, TC], F32, tag="pj", bufs=2, name=nm)
                for kt in range(KT):
                    nc.tensor.matmul(
                        pp[:], wb[:, kt * 128:(kt + 1) * 128],
                        xt_slice(c, kt, 0, TC),
                        start=(kt == 0), stop=(kt == KT - 1))
                return pp

            def attn_head(c, h, qsl, attnb, ccos):
                nkt = 4 * c + 4
                kv = h // (HL // KVL)
                pa = ps.tile([128, TC], F32, tag="pa", bufs=2, name="pa")
                # denominator accumulator (bf16 SBUF): exp tiles are
                # summed here by DVE adds; ONE ones-matmul per head at
                # the end replaces per-pair denominator matmuls on PE.
                dacc = sb.tile([128, TC], BF16, tag="dacc", bufs=1,
                               name="dacc")
                pts = {}

                def qoff(kt):
                    j = kt - 4 * c
                    return 128 * j if j >= 0 else 0

                def qk_exp(kt):
                    o = qoff(kt)
                    sps = ps.tile([128, TC], F32, tag="s", bufs=2,
                                  name="sps")
                    nc.tensor.matmul(
                        sps[:, o:],
                        ktb[:, kv * S + kt * 128:kv * S + (kt + 1) * 128],
                        qsl[:, o:], start=True, stop=True)
                    pt = sb.tile([128, TC], BF16, tag="pt", bufs=6,
                                 name="pt")
                    nc.scalar.activation(
                        pt[:, o:], sps[:, o:],
                        mybir.ActivationFunctionType.Exp, scale=SCALE)
                    if kt - 4 * c >= 0:
                        nc.gpsimd.tensor_tensor(
                            pt[:, o:o + 128], pt[:, o:o + 128], mask_sb[:],
                            mybir.AluOpType.mult)
                    pts[kt] = pt

                def pv(kt):
                    o = qoff(kt)
                    nc.tensor.matmul(
                        pa[:, o:],
                        vb[:, kt * (KVL * 128) + kv * 128:
                           kt * (KVL * 128) + (kv + 1) * 128],
                        pts[kt][:, o:],
                        start=(kt == 0), stop=(kt == nkt - 1))

                def den_merge(kt):
                    pt = pts.pop(kt)
                    if kt == 0:
                        nc.vector.tensor_copy(dacc[:], pt[:])
                    else:
                        o = qoff(kt)
                        nc.vector.tensor_tensor(
                            dacc[:, o:], dacc[:, o:], pt[:, o:],
                            mybir.AluOpType.add)

                qk_exp(0)
                qk_exp(1)
                for kt in range(2, nkt):
                    k = kt - 2
                    pv(k)
                    den_merge(k)
                    qk_exp(kt)
                pv(nkt - 2)
                den_merge(nkt - 2)
                pv(nkt - 1)
                den_merge(nkt - 1)

                pd = ps.tile([128, TC], F32, tag="s", bufs=2, name="pdn")
                nc.tensor.matmul(pd[:], ones_sb[:], dacc[:],
                                 start=True, stop=True)
                rc = sb.tile([128, TC], F32, tag="rc", bufs=1, name="rc")
                nc.vector.reciprocal_approx_fast(rc[:], pd[:])
                nc.vector.tensor_tensor(
                    attnb[:, h * TC:(h + 1) * TC], pa[:], rc[:],
                    mybir.AluOpType.mult)
                if h == 3 or h == 7:
                    ccos.append(gather_piece(attnb, h - 3))

            def chunk_cycle(c):
                """Fused proj+attention for one token chunk: K/V first,
                then per q-head [panel matmuls -> rope -> attention].
                Attention's scalar-engine exp chain overlaps the next
                head's (PE-bound) projection panel, and collective pieces
                trigger a full chunk earlier than a split proj/attn
                schedule."""
                t0 = c * TC
                wv = get_wv2(nc.gpsimd)   # post early on an idle DMA queue
                if c + 1 < NCHUNK:
                    load_xt(c + 1)        # posted now; fires as WARs clear
                for kv in range(KVL):
                    pk = proj_panel(c, 8 + kv, "pk")
                    rope(ktb[:, kv * S + t0:kv * S + t0 + TC], pk, t0)
                # direct V: out[t, d] accumulated with X^T tile stationary,
                # wv panel moving (N=256 covers both kv heads). Two token
                # tiles share one PSUM bank -- SEQUENTIAL groups only:
                # start_tensor_calc marks a whole 2KB zero-region, so
                # interleaving two live groups in one bank corrupts the
                # earlier group's accumulation.
                for tp_ in range(2):
                    pvd = ps.tile([128, TC], F32, tag="pj", bufs=2, name="pvd")
                    for half in range(2):
                        tb = 2 * tp_ + half
                        for kt in range(KT):
                            nc.tensor.matmul(
                                pvd[:, half * 256:(half + 1) * 256],
                                xt_slice(c, kt, tb * 128, tb * 128 + 128),
                                wv[:, kt * 256:(kt + 1) * 256],
                                start=(kt == 0), stop=(kt == KT - 1))
                    vt0 = 4 * c + 2 * tp_
                    nc.scalar.copy(
                        vb[:, vt0 * 256:(vt0 + 2) * 256], pvd[:])
                qtb = sb.tile([128, HL * TC], BF16, tag="qt", bufs=2,
                              name="qtb")
                attnb = sb.tile([128, HL * TC], BF16, tag="attn", bufs=1,
                                name="attnb")
                ccos = []
                for h in range(HL):
                    pq = proj_panel(c, h, "pq")
                    rope(qtb[:, h * TC:(h + 1) * TC], pq, t0)
                    attn_head(c, h, qtb[:, h * TC:(h + 1) * TC], attnb, ccos)
                return ccos

            def gather_piece(attnb, h0):
                """AllGather heads [h0, h0+4) of attnb -> cco [4*4*128, TC]."""
                cci = dr.tile([4 * 128, TC], BF16, tag="cci", bufs=8,
                              name="cci")
                cco = dr.tile([TP * 4 * 128, TC], BF16, tag="cco", bufs=8,
                              name="cco")
                nc.scalar.dma_start(
                    cci.rearrange("(h p) t -> p h t", p=128),
                    attnb.rearrange("p (h t) -> p h t", h=HL)[:, h0:h0 + 4])
                nc.gpsimd.collective_compute(
                    "AllGather", mybir.AluOpType.bypass,
                    replica_groups=GROUPS, ins=[cci[:]], outs=[cco[:]])
                return cco

            def load_af(cco, floor_ms):
                """cco piece -> SBUF [128, 16*TC] (g = r*4 + local head).

                floor_ms is a scheduler hint: the tile list-scheduler's sim
                has no model of AllGather latency, so without a floor it
                schedules af-consuming matmuls right after the collective
                trigger and the PE stalls on the real collective."""
                af = sb.tile([128, 16 * TC], BF16, tag="af", bufs=2, name="af")
                with tc.tile_wait_until(floor_ms):
                    nc.sync.dma_start(
                        af.rearrange("p (g t) -> p g t", g=16),
                        cco.rearrange("(g p) t -> p g t", p=128))
                return af

            def af_slice(afs, dt):
                """af slice for global d-tile dt (wo row-block r*8+hh)."""
                r, hh = dt // 8, dt % 8
                af = afs[hh // 4]
                slot = r * 4 + (hh % 4)
                return af[:, slot * TC:(slot + 1) * TC]

            # dts whose af slice comes from piece 0 (hh%8 < 4), then piece 1:
            # accumulating piece-0 rows first lets each panel start before
            # the second collective piece lands.
            DT_ORDER = ([dt for dt in range(KT) if dt % 8 < 4] +
                        [dt for dt in range(KT) if dt % 8 >= 4])

            def outproj(c, ccos, floors, wide=False):
                # NOTE: outproj uses dedicated "wo"/"po" pools: its
                # execution is collective-gated, and same-tag pool buffers
                # recycle in allocation order -- sharing "wb"/"pj" with a
                # later proj() would WAR-chain that proj behind the
                # collective wait. The LAST outproj (wide=True) instead
                # round-robins all four PSUM tags (everything else is done
                # by then) so all 8 panels get distinct banks and the
                # piece-0 halves all run while the last AllGather flies.
                t0 = c * TC
                afs = [load_af(ccos[0], floors[0]), load_af(ccos[1], floors[1])]
                for o in range(OC // 128):
                    wb = get_panel(12 + o, tag="wo", eng=nc.scalar)
                    ptag = ["po", "pa", "s", "pj"][o % 4] if wide else "po"
                    po = ps.tile([128, TC], F32, tag=ptag, bufs=2, name="po")
                    for i, dt in enumerate(DT_ORDER):
                        nc.tensor.matmul(
                            po[:], wb[:, dt * 128:(dt + 1) * 128],
                            af_slice(afs, dt),
                            start=(i == 0), stop=(i == KT - 1))
                    ot = sb.tile([128, TC], F32, tag="ot", bufs=1, name="ot")
                    nc.vector.tensor_copy(ot[:], po[:])
                    nc.gpsimd.dma_start(
                        out_d[o * 128:(o + 1) * 128, t0:t0 + TC], ot[:])

            # ---- schedule ----
            # Chunk-interleaved: proj(c) -> attn(c) so collective pieces
            # trigger early and the serialized AllGather chain hides
            # behind remaining matmul work. Outprojs are PE filler once
            # their afs are (estimated) complete.
            # Startup: sync queue is reserved for weight panels (first MM
            # needs panel 0); X^T goes on scalar+gpsimd queues; cos/sin
            # are not needed until the first rope's DVE ops (~20us) and
            # mask/ones not until attn0 (~120us), so they queue last.
            load_xt(0)
            nc.scalar.dma_start(cos_sb[:], cos_d[:])
            nc.scalar.dma_start(sin_sb[:], sin_d[:])
            nc.gpsimd.dma_start(mask_sb[:], mask_d[:])
            nc.gpsimd.dma_start(ones_sb[:], ones_d[:])
            # Dummy tiny AllGather to absorb the cold-start cost of the
            # collective stream during proj(0).
            dgi = dr.tile([128, 128], BF16, tag="dgi", bufs=1, name="dgi")
            dgo = dr.tile([TP * 128, 128], BF16, tag="dgo", bufs=1,
                          name="dgo")
            nc.gpsimd.dma_start(dgi[:], mask_d[:])
            nc.gpsimd.collective_compute(
                "AllGather", mybir.AluOpType.bypass,
                replica_groups=GROUPS, ins=[dgi[:]], outs=[dgo[:]])

            gath = {}
            gath[0] = chunk_cycle(0)
            gath[1] = chunk_cycle(1)
            outproj(0, gath[0], (0.15, 0.18))
            gath[2] = chunk_cycle(2)
            outproj(1, gath[1], (0.29, 0.32))
            gath[3] = chunk_cycle(3)
            outproj(2, gath[2], (0.44, 0.47))
            outproj(3, gath[3], (0.59, 0.65), wide=True)

    nc.compile()
    return nc


def _get_nc():
    if "nc" not in _BUILT:
        _BUILT["nc"] = _build()
    return _BUILT["nc"]


def _to_bf16(a):
    return np.ascontiguousarray(a.astype(ml_dtypes.bfloat16))


def _panelize(w):
    """[H, C] fp32 -> [128, KT*C] bf16 in [p, kt, c] layout."""
    c = w.shape[1]
    return _to_bf16(
        w.reshape(KT, 128, c).transpose(1, 0, 2).reshape(128, KT * c))


def kernel(hidden_states, cos, sin, wq, wk, wv, wo):
    global LAST_RESULT
    nc = _get_nc()

    hidden_states = np.asarray(hidden_states, dtype=np.float32)
    cos = np.asarray(cos, dtype=np.float32)
    sin = np.asarray(sin, dtype=np.float32)
    wq = np.asarray(wq, dtype=np.float32)
    wk = np.asarray(wk, dtype=np.float32)
    wv = np.asarray(wv, dtype=np.float32)
    wo = np.asarray(wo, dtype=np.float32)

    # host-side shard prep: X^T as [128, kt*S] bf16
    xts = []
    for b in range(B):
        xt = hidden_states[b].T.reshape(KT, 128, S).transpose(1, 0, 2)
        xts.append(_to_bf16(xt.reshape(128, KT * S)))
    cts = [np.ascontiguousarray(cos[b].T) for b in range(B)]
    sin_eff = []
    for b in range(B):
        st = np.ascontiguousarray(sin[b].T)
        se = st.copy()
        se[0:64, :] *= -1.0
        sin_eff.append(se)

    maskb = np.triu(np.ones((128, 128))).astype(ml_dtypes.bfloat16)
    onesb = np.ones((128, 128), dtype=ml_dtypes.bfloat16)

    # weight panels per tp rank: 8 wq, 2 wk, 1 double wv, 8 wo
    w_alls = []
    for r in range(TP):
        panels = []
        for h in range(HL):
            panels.append(_panelize(wq[:, r * QCOLS + h * 128:
                                       r * QCOLS + (h + 1) * 128]))
        for kv in range(KVL):
            panels.append(_panelize(wk[:, r * (KVL * 128) + kv * 128:
                                       r * (KVL * 128) + (kv + 1) * 128]))
        panels.append(_panelize(wv[:, r * (KVL * 128):
                                   (r + 1) * (KVL * 128)]))
        for o in range(OC // 128):
            panels.append(_panelize(wo[:, r * OC + o * 128:
                                       r * OC + (o + 1) * 128]))
        w_alls.append(np.concatenate(panels, axis=1))

    in_maps = []
    for core in range(N_CORES):
        b, r = core // TP, core % TP
        in_maps.append({
            "xt": xts[b],
            "w_all": w_alls[r],
            "cos_t": cts[b],
            "sin_t": sin_eff[b],
            "maskb": maskb,
            "onesb": onesb,
        })

    res = run_bass_kernel_spmd(nc, in_maps, core_ids=list(range(N_CORES)))
    LAST_RESULT = res

    out = np.empty((B, S, H), dtype=np.float32)
    for core in range(N_CORES):
        b, r = core // TP, core % TP
        out[b, :, r * OC:(r + 1) * OC] = res.results[core]["out_t"].T
    return out


# revision 25
# speedup vs baseline: 1.0894x; 1.0139x over previous
"""Llama GQA attention (B=2, S=2048, H=4096, 32 q heads / 8 kv heads, HD=128)
on 8 Trainium2 NeuronCores.

Sharding: DP=2 over batch x TP=4 over heads.
  core c: batch b = c // 4, tp rank r = c % 4
  - owns q heads [8r, 8r+8), kv heads [2r, 2r+2)
  - AllGather (bf16) of attention outputs within each 4-core batch group,
    split in two 4-head pieces triggered mid-attention
  - output projection over the full 4096 attn features for output columns
    [1024r, 1024(r+1)) -> disjoint outputs, host concatenates.

All inputs are pre-cast/pre-transposed to the on-chip layout on the HOST
(bf16 weights/X^T), so the device does zero dtype conversion and reads
every operand exactly once per use:
  xt:  [128, kt*S + t]   X^T with hidden dim split into 32 k-tiles
  w:   [128, panel*4096] weight panels (8 wq, 2 wk, 1 double-width wv,
                          8 wo), each [128 k-part, kt*cols + c]
On-chip layout is "transposed" ([feature, token]) for Q/K; V is computed
DIRECTLY in [token, dim] layout (lhsT = X^T tile stationary, wv panel
moving, N=256 covering both kv heads) -- no PE transposes needed.
Causal masking: per q-chunk c only k-tiles 0..4c+3 are touched; the 4
diagonal tiles compute only the surviving q-range (free = 512-128j) plus
one static [128,128] triangular mask on the leading square (GpSimd).
Softmax skips max-subtraction (scores are O(8), exp fits bf16). The
denominator merges each adjacent pair of exp tiles with one in-place
DVE bf16 add and runs a single ones-matmul per pair on the PE (half
the denominator matmuls); the reciprocal uses the fast DVE
approximation (18 bits).

Schedule (v2): chunk-interleaved -- proj(c) then IMMEDIATELY attn(c)
(possible because attn(c) only needs K/V chunks 0..c), so the first
AllGather piece triggers ~120us into the kernel instead of ~480us.
The 8 AllGather pieces serialize on the collective stream (~25-45us
each, ~90us for the cold first one); triggering them early hides the
entire ~350us chain behind remaining proj/attn/outproj matmul work
instead of exposing its tail. Out-projections are placed as PE filler:
op0 after attn1, op1 after attn2, op2+op3 after attn3. The af loads
carry tile_wait_until floors approximating real collective completion
so the list scheduler (whose sim has no collective-latency model) does
not hoist af-consuming matmuls ahead of available attention work.
Startup DMAs are posted xt-first across four engine queues so the
first matmul issues ~12us in instead of ~30us.
"""

import sys

for _p in ("/opt/trn_rl_repo",):
    if _p not in sys.path:
        sys.path.append(_p)

import numpy as np
import ml_dtypes

import concourse.bacc as bacc
import concourse.mybir as mybir
import concourse.tile as tile
from concourse.bass_utils import run_bass_kernel_spmd

F32 = mybir.dt.float32
BF16 = mybir.dt.bfloat16

B, S, H = 2, 2048, 4096
NH, NKV, HD = 32, 8, 128
N_CORES = 8
TP = 4
GROUPS = [[0, 1, 2, 3], [4, 5, 6, 7]]

HL = NH // TP          # 8 local q heads
KVL = NKV // TP        # 2 local kv heads
QCOLS = HL * HD        # 1024 local q cols
OC = H // TP           # 1024 local out cols

TC = 512               # token chunk (= one attention q-block)
NCHUNK = S // TC       # 4
KT = H // 128          # 32 contraction tiles
SCALE = float(HD ** -0.5)
NPANEL = 20            # 8 wq + 2 wk + 2 (double wv) + 8 wo
PANW = KT * 128        # 4096 free cols per standard weight panel

LAST_RESULT = None
_BUILT = {}


def _build():
    nc = bacc.Bacc("TRN2", debug=False, num_devices=N_CORES)

    xt_d = nc.dram_tensor("xt", [128, KT * S], BF16, kind="ExternalInput").ap()
    w_d = nc.dram_tensor("w_all", [128, NPANEL * PANW], BF16,
                         kind="ExternalInput").ap()
    cos_d = nc.dram_tensor("cos_t", [HD, S], F32, kind="ExternalInput").ap()
    sin_d = nc.dram_tensor("sin_t", [HD, S], F32, kind="ExternalInput").ap()
    mask_d = nc.dram_tensor("maskb", [128, 128], BF16, kind="ExternalInput").ap()
    ones_d = nc.dram_tensor("onesb", [128, 128], BF16, kind="ExternalInput").ap()
    out_d = nc.dram_tensor("out_t", [OC, S], F32, kind="ExternalOutput").ap()

    with tile.TileContext(nc) as tc:
        with tc.tile_pool(name="sb", bufs=1) as sb, \
             tc.tile_pool(name="ps", bufs=1, space="PSUM") as ps, \
             tc.tile_pool(name="dr", bufs=1, space="DRAM") as dr:

            # ---- persistent tiles ----
            cos_sb = sb.tile([HD, S], F32)
            sin_sb = sb.tile([HD, S], F32)
            mask_sb = sb.tile([128, 128], BF16)
            ones_sb = sb.tile([128, 128], BF16)
            ktb = sb.tile([128, KVL * S], BF16)            # roped K^T [d, kv*S+t]
            vb = sb.tile([128, (S // 128) * KVL * 128], BF16)  # V [t, tt*256+d]

            _XTB = {}

            def load_xt(c, engs=None):
                """X^T chunk c -> four SBUF quarter-tiles [128, 8*TC].

                Quarters with bufs=7 let chunk c+1's loads start while
                chunk c is still being consumed (a monolithic bufs=1 tile
                made the load wait for proj(c)'s LAST matmul, stalling the
                in-order PE queue ~20us per chunk)."""
                engs = engs or [nc.scalar, nc.gpsimd, nc.scalar, nc.gpsimd]
                quarters = []
                for qf in range(4):
                    xth = sb.tile([128, (KT // 4) * TC], BF16, tag="xtb",
                                  bufs=7, name=f"xtb{c}_{qf}")
                    k0 = qf * (KT // 4)
                    engs[qf].dma_start(
                        xth.rearrange("p (kt t) -> p kt t", t=TC),
                        xt_d.rearrange("p (kt t) -> p kt t", t=S)
                        [:, k0:k0 + KT // 4, c * TC:(c + 1) * TC])
                    quarters.append(xth)
                _XTB[c] = quarters

            def xt_slice(c, kt, lo, hi):
                """X^T [128, lo:hi] slice of k-tile kt for chunk c."""
                xth = _XTB[c][kt // 8]
                base = (kt % 8) * TC
                return xth[:, base + lo:base + hi]

            def get_panel(idx, eng=None, tag="wb", split=2):
                wb = sb.tile([128, PANW], BF16, tag=tag, bufs=2, name=tag)
                step = PANW // split
                for si in range(split):
                    (eng or nc.sync).dma_start(
                        wb[:, si * step:(si + 1) * step],
                        w_d[:, idx * PANW + si * step:
                            idx * PANW + (si + 1) * step])
                return wb

            def get_wv2(eng=None):
                """Double-width V panel [128, kt*256 + d] at slot 10."""
                wb = sb.tile([128, 2 * PANW], BF16, tag="wv2", bufs=1,
                             name="wv2")
                (eng or nc.sync).dma_start(
                    wb[:], w_d[:, 10 * PANW:12 * PANW])
                return wb

            def rope(dst, pq, t0):
                """dst (bf16 [128, TC]) = rope of pq (fp32 PSUM [128, TC])."""
                qf = sb.tile([128, TC], F32, tag="qf", bufs=1)
                nc.vector.tensor_copy(qf[:], pq[:])
                qs = sb.tile([128, TC], F32, tag="qs", bufs=1)
                nc.sync.dma_start(qs[0:64, :], qf[64:128, :])
                nc.sync.dma_start(qs[64:128, :], qf[0:64, :])
                nc.vector.tensor_tensor(
                    qf[:], qf[:], cos_sb[:, t0:t0 + TC], mybir.AluOpType.mult)
                nc.vector.tensor_tensor(
                    qs[:], qs[:], sin_sb[:, t0:t0 + TC], mybir.AluOpType.mult)
                nc.vector.tensor_tensor(dst, qf[:], qs[:], mybir.AluOpType.add)

            def proj_panel(c, idx, nm):
                wb = get_panel(idx)
                pp = ps.tile([128, TC], F32, tag="pj", bufs=2, name=nm)
                for kt in range(KT):
                    nc.tensor.matmul(
                        pp[:], wb[:, kt * 128:(kt + 1) * 128],
                        xt_slice(c, kt, 0, TC),
                        start=(kt == 0), stop=(kt == KT - 1))
                return pp

            def attn_head(c, h, qsl, attnb, ccos):
                nkt = 4 * c + 4
                kv = h // (HL // KVL)
                pa = ps.tile([128, TC], F32, tag="pa", bufs=2, name="pa")
                # denominator accumulator (bf16 SBUF): exp tiles are
                # summed here by DVE adds; ONE ones-matmul per head at
                # the end replaces per-pair denominator matmuls on PE.
                dacc = sb.tile([128, TC], BF16, tag="dacc", bufs=1,
                               name="dacc")
                pts = {}

                def qoff(kt):
                    j = kt - 4 * c
                    return 128 * j if j >= 0 else 0

                def qk_exp(kt):
                    o = qoff(kt)
                    sps = ps.tile([128, TC], F32, tag="s", bufs=2,
                                  name="sps")
                    nc.tensor.matmul(
                        sps[:, o:],
                        ktb[:, kv * S + kt * 128:kv * S + (kt + 1) * 128],
                        qsl[:, o:], start=True, stop=True)
                    pt = sb.tile([128, TC], BF16, tag="pt", bufs=6,
                                 name="pt")
                    nc.scalar.activation(
                        pt[:, o:], sps[:, o:],
                        mybir.ActivationFunctionType.Exp, scale=SCALE)
                    if kt - 4 * c >= 0:
                        nc.gpsimd.tensor_tensor(
                            pt[:, o:o + 128], pt[:, o:o + 128], mask_sb[:],
                            mybir.AluOpType.mult)
                    pts[kt] = pt

                def pv(kt):
                    o = qoff(kt)
                    nc.tensor.matmul(
                        pa[:, o:],
                        vb[:, kt * (KVL * 128) + kv * 128:
                           kt * (KVL * 128) + (kv + 1) * 128],
                        pts[kt][:, o:],
                        start=(kt == 0), stop=(kt == nkt - 1))

                def den_merge(kt):
                    pt = pts.pop(kt)
                    if kt == 0:
                        nc.vector.tensor_copy(dacc[:], pt[:])
                    else:
                        o = qoff(kt)
                        nc.vector.tensor_tensor(
                            dacc[:, o:], dacc[:, o:], pt[:, o:],
                            mybir.AluOpType.add)

                qk_exp(0)
                qk_exp(1)
                for kt in range(2, nkt):
                    k = kt - 2
                    pv(k)
                    den_merge(k)
                    qk_exp(kt)
                pv(nkt - 2)
                den_merge(nkt - 2)
                pv(nkt - 1)
                den_merge(nkt - 1)

                pd = ps.tile([128, TC], F32, tag="s", bufs=2, name="pdn")
                nc.tensor.matmul(pd[:], ones_sb[:], dacc[:],
                                 start=True, stop=True)
                rc = sb.tile([128, TC], F32, tag="rc", bufs=1, name="rc")
                nc.vector.reciprocal_approx_fast(rc[:], pd[:])
                nc.vector.tensor_tensor(
                    attnb[:, h * TC:(h + 1) * TC], pa[:], rc[:],
                    mybir.AluOpType.mult)
                if h % 2 == 1:
                    ccos.append(gather_piece(attnb, h - 1))

            def chunk_cycle(c):
                """Fused proj+attention for one token chunk: K/V first,
                then per q-head [panel matmuls -> rope -> attention].
                Attention's scalar-engine exp chain overlaps the next
                head's (PE-bound) projection panel, and collective pieces
                trigger a full chunk earlier than a split proj/attn
                schedule."""
                t0 = c * TC
                wv = get_wv2(nc.gpsimd)   # post early on an idle DMA queue
                if c + 1 < NCHUNK:
                    load_xt(c + 1)        # posted now; fires as WARs clear
                for kv in range(KVL):
                    pk = proj_panel(c, 8 + kv, "pk")
                    rope(ktb[:, kv * S + t0:kv * S + t0 + TC], pk, t0)
                # direct V: out[t, d] accumulated with X^T tile stationary,
                # wv panel moving (N=256 covers both kv heads). Two token
                # tiles share one PSUM bank -- SEQUENTIAL groups only:
                # start_tensor_calc marks a whole 2KB zero-region, so
                # interleaving two live groups in one bank corrupts the
                # earlier group's accumulation.
                for tp_ in range(2):
                    pvd = ps.tile([128, TC], F32, tag="pj", bufs=2, name="pvd")
                    for half in range(2):
                        tb = 2 * tp_ + half
                        for kt in range(KT):
                            nc.tensor.matmul(
                                pvd[:, half * 256:(half + 1) * 256],
                                xt_slice(c, kt, tb * 128, tb * 128 + 128),
                                wv[:, kt * 256:(kt + 1) * 256],
                                start=(kt == 0), stop=(kt == KT - 1))
                    vt0 = 4 * c + 2 * tp_
                    nc.vector.tensor_copy(
                        vb[:, vt0 * 256:(vt0 + 2) * 256], pvd[:])
                qtb = sb.tile([128, HL * TC], BF16, tag="qt", bufs=2,
                              name="qtb")
                attnb = sb.tile([128, HL * TC], BF16, tag="attn", bufs=1,
                                name="attnb")
                ccos = []
                for h in range(HL):
                    pq = proj_panel(c, h, "pq")
                    rope(qtb[:, h * TC:(h + 1) * TC], pq, t0)
                    attn_head(c, h, qtb[:, h * TC:(h + 1) * TC], attnb, ccos)
                return ccos

            def gather_piece(attnb, h0):
                """AllGather heads [h0, h0+2) of attnb -> cco [4*2*128, TC].

                Four small pieces per chunk (vs two) so the tail piece of
                the last chunk is small and the out-projection can consume
                earlier pieces while later ones are still in flight."""
                cci = dr.tile([2 * 128, TC], BF16, tag="cci", bufs=16,
                              name="cci")
                cco = dr.tile([TP * 2 * 128, TC], BF16, tag="cco", bufs=16,
                              name="cco")
                nc.scalar.dma_start(
                    cci.rearrange("(h p) t -> p h t", p=128),
                    attnb.rearrange("p (h t) -> p h t", h=HL)[:, h0:h0 + 2])
                nc.gpsimd.collective_compute(
                    "AllGather", mybir.AluOpType.bypass,
                    replica_groups=GROUPS, ins=[cci[:]], outs=[cco[:]])
                return cco

            def load_af(cco, floor_ms):
                """cco piece -> SBUF [128, 8*TC] (g = r*2 + local head).

                floor_ms is a scheduler hint: the tile list-scheduler's sim
                has no model of AllGather latency, so without a floor it
                schedules af-consuming matmuls right after the collective
                trigger and the PE stalls on the real collective."""
                af = sb.tile([128, 8 * TC], BF16, tag="af", bufs=4, name="af")
                with tc.tile_wait_until(floor_ms):
                    nc.sync.dma_start(
                        af.rearrange("p (g t) -> p g t", g=8),
                        cco.rearrange("(g p) t -> p g t", p=128))
                return af

            def af_slice(afs, dt):
                """af slice for global d-tile dt (wo row-block r*8+hh)."""
                r, hh = dt // 8, dt % 8
                af = afs[hh // 2]
                slot = r * 2 + (hh % 2)
                return af[:, slot * TC:(slot + 1) * TC]

            # dts whose af slice comes from piece 0 (hh%8 < 4), then piece 1:
            # accumulating piece-0 rows first lets each panel start before
            # the second collective piece lands.
            DT_ORDER = [dt for p in range(4)
                        for dt in range(KT) if (dt % 8) // 2 == p]

            def outproj(c, ccos, floors, wide=False):
                # NOTE: outproj uses dedicated "wo"/"po" pools: its
                # execution is collective-gated, and same-tag pool buffers
                # recycle in allocation order -- sharing "wb"/"pj" with a
                # later proj() would WAR-chain that proj behind the
                # collective wait. The LAST outproj (wide=True) instead
                # round-robins all four PSUM tags (everything else is done
                # by then) so all 8 panels get distinct banks and the
                # piece-0 halves all run while the last AllGather flies.
                t0 = c * TC
                afs = [load_af(cco, fl) for cco, fl in zip(ccos, floors)]
                for o in range(OC // 128):
                    wb = get_panel(12 + o, tag="wo", eng=nc.gpsimd)
                    ptag = ["po", "pa", "s", "pj"][o % 4] if wide else "po"
                    po = ps.tile([128, TC], F32, tag=ptag, bufs=2, name="po")
                    for i, dt in enumerate(DT_ORDER):
                        nc.tensor.matmul(
                            po[:], wb[:, dt * 128:(dt + 1) * 128],
                            af_slice(afs, dt),
                            start=(i == 0), stop=(i == KT - 1))
                    ot = sb.tile([128, TC], F32, tag="ot", bufs=1, name="ot")
                    nc.vector.tensor_copy(ot[:], po[:])
                    nc.gpsimd.dma_start(
                        out_d[o * 128:(o + 1) * 128, t0:t0 + TC], ot[:])

            # ---- schedule ----
            # Chunk-interleaved: proj(c) -> attn(c) so collective pieces
            # trigger early and the serialized AllGather chain hides
            # behind remaining matmul work. Outprojs are PE filler once
            # their afs are (estimated) complete.
            # Startup: sync queue is reserved for weight panels (first MM
            # needs panel 0); X^T goes on scalar+gpsimd queues; cos/sin
            # are not needed until the first rope's DVE ops (~20us) and
            # mask/ones not until attn0 (~120us), so they queue last.
            load_xt(0)
            nc.scalar.dma_start(cos_sb[:], cos_d[:])
            nc.scalar.dma_start(sin_sb[:], sin_d[:])
            nc.gpsimd.dma_start(mask_sb[:], mask_d[:])
            nc.gpsimd.dma_start(ones_sb[:], ones_d[:])
            # Dummy tiny AllGather to absorb the cold-start cost of the
            # collective stream during proj(0).
            dgi = dr.tile([128, 128], BF16, tag="dgi", bufs=1, name="dgi")
            dgo = dr.tile([TP * 128, 128], BF16, tag="dgo", bufs=1,
                          name="dgo")
            nc.gpsimd.dma_start(dgi[:], mask_d[:])
            nc.gpsimd.collective_compute(
                "AllGather", mybir.AluOpType.bypass,
                replica_groups=GROUPS, ins=[dgi[:]], outs=[dgo[:]])

            gath = {}
            gath[0] = chunk_cycle(0)
            gath[1] = chunk_cycle(1)
            outproj(0, gath[0], (0.12, 0.15, 0.18, 0.21))
            gath[2] = chunk_cycle(2)
            outproj(1, gath[1], (0.26, 0.30, 0.34, 0.38))
            gath[3] = chunk_cycle(3)
            outproj(2, gath[2], (0.44, 0.48, 0.52, 0.56))
            outproj(3, gath[3], (0.62, 0.68, 0.73, 0.78), wide=True)

    nc.compile()
    return nc


def _get_nc():
    if "nc" not in _BUILT:
        _BUILT["nc"] = _build()
    return _BUILT["nc"]


def _to_bf16(a):
    return np.ascontiguousarray(a.astype(ml_dtypes.bfloat16))


def _panelize(w):
    """[H, C] fp32 -> [128, KT*C] bf16 in [p, kt, c] layout."""
    c = w.shape[1]
    return _to_bf16(
        w.reshape(KT, 128, c).transpose(1, 0, 2).reshape(128, KT * c))


def kernel(hidden_states, cos, sin, wq, wk, wv, wo):
    global LAST_RESULT
    nc = _get_nc()

    hidden_states = np.asarray(hidden_states, dtype=np.float32)
    cos = np.asarray(cos, dtype=np.float32)
    sin = np.asarray(sin, dtype=np.float32)
    wq = np.asarray(wq, dtype=np.float32)
    wk = np.asarray(wk, dtype=np.float32)
    wv = np.asarray(wv, dtype=np.float32)
    wo = np.asarray(wo, dtype=np.float32)

    # host-side shard prep: X^T as [128, kt*S] bf16
    xts = []
    for b in range(B):
        xt = hidden_states[b].T.reshape(KT, 128, S).transpose(1, 0, 2)
        xts.append(_to_bf16(xt.reshape(128, KT * S)))
    cts = [np.ascontiguousarray(cos[b].T) for b in range(B)]
    sin_eff = []
    for b in range(B):
        st = np.ascontiguousarray(sin[b].T)
        se = st.copy()
        se[0:64, :] *= -1.0
        sin_eff.append(se)

    maskb = np.triu(np.ones((128, 128))).astype(ml_dtypes.bfloat16)
    onesb = np.ones((128, 128), dtype=ml_dtypes.bfloat16)

    # weight panels per tp rank: 8 wq, 2 wk, 1 double wv, 8 wo
    w_alls = []
    for r in range(TP):
        panels = []
        for h in range(HL):
            panels.append(_panelize(wq[:, r * QCOLS + h * 128:
                                       r * QCOLS + (h + 1) * 128]))
        for kv in range(KVL):
            panels.append(_panelize(wk[:, r * (KVL * 128) + kv * 128:
                                       r * (KVL * 128) + (kv + 1) * 128]))
        panels.append(_panelize(wv[:, r * (KVL * 128):
                                   (r + 1) * (KVL * 128)]))
        for o in range(OC // 128):
            panels.append(_panelize(wo[:, r * OC + o * 128:
                                       r * OC + (o + 1) * 128]))
        w_alls.append(np.concatenate(panels, axis=1))

    in_maps = []
    for core in range(N_CORES):
        b, r = core // TP, core % TP
        in_maps.append({
            "xt": xts[b],
            "w_all": w_alls[r],
            "cos_t": cts[b],
            "sin_t": sin_eff[b],
            "maskb": maskb,
            "onesb": onesb,
        })

    res = run_bass_kernel_spmd(nc, in_maps, core_ids=list(range(N_CORES)))
    LAST_RESULT = res

    out = np.empty((B, S, H), dtype=np.float32)
    for core in range(N_CORES):
        b, r = core // TP, core % TP
        out[b, :, r * OC:(r + 1) * OC] = res.results[core]["out_t"].T
    return out
